# revision 13
# baseline (speedup 1.0000x reference)
"""STFT (Conv1D-style) Bass kernel for Trainium2, 8 NeuronCores.

Radix-2 decimation-in-time restructure (2x fewer MACs than direct DFT):
  - Split each frame's 1024 samples into even/odd streams. With window and
    twiddles folded into the bases (= even/odd rows of the original windowed
    DFT basis, k=0..255 only):
      E(k) = sum_m x[2m] w[2m] W^(2mk),  O(k) = sum_m x[2m+1] w[2m+1] W^((2m+1)k)
  - Then X_k = E+O for k=0..255 and X_{512-k} = conj(E-O):
    combine is ONE psum->sbuf copy + add + sub per tile; host negates the
    upper block's imag half and computes the single missing freq k=256.
  - Layout: freqs on psum partitions (2 blocks of 128), frames on the
    moving dim. HOP=256 = 2*128, so stream samples of frame t are
    xs[128t .. 128t+511]; blocked [128 x 1879] layouts make every frame
    tile a contiguous column slice (no parity split needed).
  - Data-parallel over batch: 2 signals per core; bf16 operands/outputs
    (fp32 psum accumulate), host upcasts. Combine alternates between a
    DVE path (ACT copies O, DVE adds from psum) and a GP path (ACT copies
    both, GpSimd adds in sbuf) to spread load over all vector engines.
"""

import numpy as np
import ml_dtypes

N_FFT = 1024
HOP = 256
B = 16
T = 480000
F = N_FFT // 2 + 1          # 513
PAD = N_FFT // 2            # 512
XP_LEN = T + 2 * PAD        # 481024
NF = (XP_LEN - N_FFT) // HOP + 1   # 1876 frames
NCORES = 8
B_PER_CORE = B // NCORES    # 2
NBC = 1879                  # stream-layout columns = 240512 / 128
XS_LEN = XP_LEN // 2        # 240512 samples per stream
FT_SIZES = [512, 512, 512, 340]     # frame tiles (sum = 1876)
HOST_KS = [256]             # freqs computed on host BLAS

_CACHE = {}


def _build_nc(repeat=1):
    import concourse.mybir as mybir
    import concourse.tile as tile
    from concourse import bacc

    idt = mybir.dt.bfloat16
    f32 = mybir.dt.float32
    add = mybir.AluOpType.add
    sub = mybir.AluOpType.subtract

    nc = bacc.Bacc("TRN2", target_bir_lowering=False, debug=False,
                   num_devices=NCORES)
    sig = nc.dram_tensor("sig", [128, B_PER_CORE, 2, NBC], idt,
                         kind="ExternalInput")
    basis = nc.dram_tensor("basis", [128, 32, 128], idt,
                           kind="ExternalInput")
    out = nc.dram_tensor("out", [B_PER_CORE, 2, 2, 128, 2, NF], idt,
                         kind="ExternalOutput")

    with tile.TileContext(nc) as tc:
        with (
            tc.tile_pool(name="sigp", bufs=min(repeat, 2)) as sigp,
            tc.tile_pool(name="basp", bufs=min(repeat, 2)) as basp,
            tc.tile_pool(name="intp", bufs=2) as intp,
            tc.tile_pool(name="outp", bufs=3) as outp,
            tc.tile_pool(name="ps", bufs=2, space="PSUM") as psp,
        ):
            for _rep in range(repeat):
                sg = sigp.tile([128, B_PER_CORE, 2, NBC], idt,
                               name="sg", tag="sg")
                bs = basp.tile([128, 32, 128], idt, name="bs", tag="bs")
                # Prologue order: unit 0 only needs sig(b0) cols 0..515 and
                # the g=0 basis half; issue those first so the first matmul
                # gates on ~0.8MB instead of the full 2.9MB input.
                nc.sync.dma_start(sg[:, 0, 0, 0:520], sig[:, 0, 0, 0:520])
                nc.sync.dma_start(sg[:, 0, 1, 0:520], sig[:, 0, 1, 0:520])
                nc.sync.dma_start(bs[:, 0:16], basis[:, 0:16])
                nc.sync.dma_start(bs[:, 16:32], basis[:, 16:32])
                nc.sync.dma_start(sg[:, 0, 0, 520:], sig[:, 0, 0, 520:])
                nc.sync.dma_start(sg[:, 0, 1, 520:], sig[:, 0, 1, 520:])
                for b in range(1, B_PER_CORE):
                    for s in range(2):
                        nc.sync.dma_start(sg[:, b, s], sig[:, b, s])

                unit = 0
                for b in range(B_PER_CORE):
                    for ft, N in enumerate(FT_SIZES):
                        f0 = 512 * ft
                        for g in range(2):
                            # E/O psum tiles [128, 2(comp), 512]
                            E = psp.tile([128, 2, 512], f32, tag="E", name="E")
                            O = psp.tile([128, 2, 512], f32, tag="O", name="O")
                            for s, pt in ((0, E), (1, O)):
                                for comp in range(2):
                                    po = pt[:, comp, 0:N]
                                    for q in range(4):
                                        ch = g * 16 + (s * 2 + comp) * 4 + q
                                        nc.tensor.matmul(
                                            po, bs[:, ch, :],
                                            sg[:, b, s, f0 + q:f0 + q + N],
                                            start=(q == 0), stop=(q == 3),
                                        )
                            Ev, Ov = E[:, :, 0:N], O[:, :, 0:N]
                            xl = outp.tile([128, 2, 512], idt, tag="xl",
                                           name="xl")[:, :, 0:N]
                            xu = outp.tile([128, 2, 512], idt, tag="xu",
                                           name="xu")[:, :, 0:N]
                            if True:  # DVE path only: GP tensor_tensor is too
                                      # slow (2.3us/op) and stalls the psum
                                      # ping-pong behind the matmuls.
                                co = intp.tile([128, 2, 512], f32, tag="co",
                                               name="co")[:, :, 0:N]
                                nc.scalar.copy(co, Ov)
                                nc.vector.tensor_tensor(xl, Ev, co, add)
                                nc.vector.tensor_tensor(xu, Ev, co, sub)
                            else:
                                ce = intp.tile([128, 2, 512], f32, tag="ce",
                                               name="ce")[:, :, 0:N]
                                co = intp.tile([128, 2, 512], f32, tag="co2",
                                               name="co2")[:, :, 0:N]
                                nc.scalar.copy(ce, Ev)
                                nc.scalar.copy(co, Ov)
                                nc.gpsimd.tensor_tensor(xl, ce, co, add)
                                nc.gpsimd.tensor_tensor(xu, ce, co, sub)
                            # out-DMAs on the (otherwise idle) gpsimd queue:
                            # keeps the sync queue free for inputs and the
                            # scalar/vector engines unblocked.
                            nc.gpsimd.dma_start(
                                out[b, 0, g, :, :, f0:f0 + N], xl)
                            nc.gpsimd.dma_start(
                                out[b, 1, g, :, :, f0:f0 + N], xu)
                            unit += 1

    nc.compile()
    return nc


def _host_prep(x, window):
    x = np.asarray(x, dtype=np.float32)
    window = np.asarray(window, dtype=np.float32)
    xp = np.pad(x, ((0, 0), (PAD, PAD)), mode="reflect")

    # stream layouts [B, 2(stream), 128, NBC]
    lay = np.empty((B, 2, 128, NBC), np.float32)
    for s in range(2):
        xs = xp[:, s::2]                              # [B, 240512]
        lay[:, s] = xs.reshape(B, NBC, 128).transpose(0, 2, 1)
    lay_bf = lay.astype(ml_dtypes.bfloat16)

    # Windowed DFT basis, fp32 angles to match the reference math.
    k = np.arange(F, dtype=np.float32)[:, None]
    n = np.arange(N_FFT, dtype=np.float32)[None, :]
    ang = np.float32(2.0 * np.pi / N_FFT) * k * n
    cosk = np.cos(ang) * window[None, :]              # [F, 1024]
    sink = -np.sin(ang) * window[None, :]
    Bre = cosk.T.astype(np.float32)                   # [1024, F]
    Bim = sink.T.astype(np.float32)

    # chunk = g*16 + (s*2 + comp)*4 + q ; rows n = 2*(128q+p)+s
    bas_host = np.empty((128, 32, 128), np.float32)
    p = np.arange(128)
    for s in range(2):
        for comp in range(2):
            src = Bre if comp == 0 else Bim
            for g in range(2):
                for q in range(4):
                    ch = g * 16 + (s * 2 + comp) * 4 + q
                    bas_host[:, ch, :] = src[2 * (128 * q + p) + s,
                                             128 * g:128 * g + 128]
    bas_bf = bas_host.astype(ml_dtypes.bfloat16)

    # host freq k=256 (fp32 BLAS)
    hb = np.stack([Bre[:, 256], Bim[:, 256]], axis=1)  # [1024, 2]
    hout = np.empty((B, NF, 2), np.float32)
    for b in range(B):
        frames = np.lib.stride_tricks.as_strided(
            xp[b], (NF, N_FFT), (HOP * 4, 4))
        hout[b] = frames @ hb

    in_maps = []
    for core in range(NCORES):
        bsl = slice(B_PER_CORE * core, B_PER_CORE * (core + 1))
        sc = np.ascontiguousarray(lay_bf[bsl].transpose(2, 0, 1, 3))
        in_maps.append({"sig": sc, "basis": bas_bf})
    return in_maps, hout


def _assemble(results, prep):
    _, hout = prep
    dev = np.concatenate(
        [np.asarray(results.results[c]["out"]) for c in range(NCORES)],
        axis=0).astype(np.float32)            # [B, 2lu, 2g, 128, 2comp, NF]
    out = np.empty((B, NF, F, 2), np.float32)
    j = np.arange(128)
    for g in range(2):
        lo = dev[:, 0, g].transpose(0, 3, 1, 2)       # [B, NF, 128, 2]
        out[:, :, 128 * g:128 * g + 128, :] = lo
        up = dev[:, 1, g].transpose(0, 3, 1, 2)       # [B, NF, 128, 2]
        ks = 512 - (128 * g + j)                      # descending freqs
        out[:, :, ks, 0] = up[:, :, :, 0]
        out[:, :, ks, 1] = -up[:, :, :, 1]
    out[:, :, 256, 0] = hout[:, :, 0]
    out[:, :, 256, 1] = hout[:, :, 1]
    return out


def kernel(x, window):
    from concourse.bass_utils import run_bass_kernel_spmd

    if "nc" not in _CACHE:
        _CACHE["nc"] = _build_nc()
    nc = _CACHE["nc"]

    prep = _host_prep(np.asarray(x), np.asarray(window))
    res = run_bass_kernel_spmd(nc, prep[0], core_ids=list(range(NCORES)),
                               trace=False)
    return _assemble(res, prep)


# revision 29
# speedup vs baseline: 1.2639x; 1.2639x over previous
"""STFT (Conv1D-style) Bass kernel for Trainium2, 8 NeuronCores.

Radix-2 decimation-in-time restructure (2x fewer MACs than direct DFT):
  - Split each frame's 1024 samples into even/odd streams. With window and
    twiddles folded into the bases (= even/odd rows of the original windowed
    DFT basis, k=0..255 only):
      E(k) = sum_m x[2m] w[2m] W^(2mk),  O(k) = sum_m x[2m+1] w[2m+1] W^((2m+1)k)
  - Then X_k = E+O for k=0..255 and X_{512-k} = conj(E-O):
    combine is ONE psum->sbuf copy + add + sub per tile; host negates the
    upper block's imag half and computes the single missing freq k=256.
  - Layout: freqs on psum partitions (2 blocks of 128), frames on the
    moving dim. HOP=256 = 2*128, so stream samples of frame t are
    xs[128t .. 128t+511]; blocked [128 x 1879] layouts make every frame
    tile a contiguous column slice (no parity split needed).
  - Data-parallel over batch: 2 signals per core; bf16 operands/outputs
    (fp32 psum accumulate), host upcasts. Combine: ACT copies O to sbuf
    (only DVE/ACT can read psum on TRN2), DVE does E+co and E-co; both
    fit under the tensor engine's ~55us with no psum ping-pong stalls.
"""

import numpy as np
import ml_dtypes

N_FFT = 1024
HOP = 256
B = 16
T = 480000
F = N_FFT // 2 + 1          # 513
PAD = N_FFT // 2            # 512
XP_LEN = T + 2 * PAD        # 481024
NF = (XP_LEN - N_FFT) // HOP + 1   # 1876 frames
NCORES = 8
B_PER_CORE = B // NCORES    # 2
NBC = 1879                  # stream-layout columns = 240512 / 128
XS_LEN = XP_LEN // 2        # 240512 samples per stream
FT_SIZES = [512, 512, 512, 340]     # frame tiles (sum = 1876)
HOST_KS = [256]             # freqs computed on host BLAS

_CACHE = {}


def _build_nc(repeat=1):
    import concourse.mybir as mybir
    import concourse.tile as tile
    from concourse import bacc

    idt = mybir.dt.bfloat16
    f32 = mybir.dt.float32
    add = mybir.AluOpType.add
    sub = mybir.AluOpType.subtract

    nc = bacc.Bacc("TRN2", target_bir_lowering=False, debug=False,
                   num_devices=NCORES)
    sig = nc.dram_tensor("sig", [128, B_PER_CORE, 2, NBC], idt,
                         kind="ExternalInput")
    basis = nc.dram_tensor("basis", [128, 32, 128], idt,
                           kind="ExternalInput")
    out = nc.dram_tensor("out", [B_PER_CORE, 2, 2, 128, 2, NF], idt,
                         kind="ExternalOutput")

    with tile.TileContext(nc) as tc:
        with (
            tc.tile_pool(name="sigp", bufs=min(repeat, 2)) as sigp,
            tc.tile_pool(name="basp", bufs=min(repeat, 2)) as basp,
            tc.tile_pool(name="intp", bufs=2) as intp,
            tc.tile_pool(name="outp", bufs=3) as outp,
            tc.tile_pool(name="ps", bufs=2, space="PSUM") as psp,
        ):
            for _rep in range(repeat):
                sg = sigp.tile([128, B_PER_CORE, 2, NBC], idt,
                               name="sg", tag="sg")
                bs = basp.tile([128, 32, 128], idt, name="bs", tag="bs")
                # Inputs on the three DMA-capable queues (sync/scalar/
                # gpsimd, ~180GB/s each): the pieces gating unit 0 (g=0
                # basis halves + the first 516 columns of b0's streams)
                # land concurrently in ~2us; the rest follows on the same
                # queues well before its consumers.
                nc.sync.dma_start(bs[:, 0:8], basis[:, 0:8])
                nc.scalar.dma_start(sg[:, 0, 0, 0:520], sig[:, 0, 0, 0:520])
                nc.gpsimd.dma_start(sg[:, 0, 1, 0:520], sig[:, 0, 1, 0:520])
                nc.scalar.dma_start(bs[:, 8:16], basis[:, 8:16])
                nc.sync.dma_start(bs[:, 16:32], basis[:, 16:32])
                nc.scalar.dma_start(sg[:, 0, 0, 520:], sig[:, 0, 0, 520:])
                nc.gpsimd.dma_start(sg[:, 0, 1, 520:], sig[:, 0, 1, 520:])
                for b in range(1, B_PER_CORE):
                    nc.scalar.dma_start(sg[:, b, 0], sig[:, b, 0])
                    nc.gpsimd.dma_start(sg[:, b, 1], sig[:, b, 1])

                unit = 0
                for b in range(B_PER_CORE):
                    for ft, N in enumerate(FT_SIZES):
                        f0 = 512 * ft
                        for g in range(2):
                            # E/O psum tiles [128, 2(comp), 512]
                            E = psp.tile([128, 2, 512], f32, tag="E", name="E")
                            O = psp.tile([128, 2, 512], f32, tag="O", name="O")
                            for s, pt in ((0, E), (1, O)):
                                for comp in range(2):
                                    po = pt[:, comp, 0:N]
                                    for q in range(4):
                                        ch = g * 16 + (s * 2 + comp) * 4 + q
                                        nc.tensor.matmul(
                                            po, bs[:, ch, :],
                                            sg[:, b, s, f0 + q:f0 + q + N],
                                            start=(q == 0), stop=(q == 3),
                                        )
                            Ev, Ov = E[:, :, 0:N], O[:, :, 0:N]
                            xl = outp.tile([128, 2, 512], idt, tag="xl",
                                           name="xl")[:, :, 0:N]
                            xu = outp.tile([128, 2, 512], idt, tag="xu",
                                           name="xu")[:, :, 0:N]
                            co = intp.tile([128, 2, 512], f32, tag="co",
                                           name="co")[:, :, 0:N]
                            nc.scalar.copy(co, Ov)
                            nc.vector.tensor_tensor(xl, Ev, co, add)
                            nc.vector.tensor_tensor(xu, Ev, co, sub)
                            # out-DMAs alternate between the gpsimd and sync
                            # queues (halves the end-of-kernel data drain).
                            # Not scalar: a descriptor stall there would
                            # delay the ACT copies feeding the combine.
                            eng_l = nc.gpsimd if unit % 2 == 0 else nc.sync
                            eng_u = nc.sync if unit % 2 == 0 else nc.gpsimd
                            eng_l.dma_start(
                                out[b, 0, g, :, :, f0:f0 + N], xl)
                            eng_u.dma_start(
                                out[b, 1, g, :, :, f0:f0 + N], xu)
                            unit += 1

    nc.compile()
    return nc


def _host_prep(x, window):
    x = np.asarray(x, dtype=np.float32)
    window = np.asarray(window, dtype=np.float32)
    xp = np.pad(x, ((0, 0), (PAD, PAD)), mode="reflect")

    # stream layouts [B, 2(stream), 128, NBC]
    lay = np.empty((B, 2, 128, NBC), np.float32)
    for s in range(2):
        xs = xp[:, s::2]                              # [B, 240512]
        lay[:, s] = xs.reshape(B, NBC, 128).transpose(0, 2, 1)
    lay_bf = lay.astype(ml_dtypes.bfloat16)

    # Windowed DFT basis, fp32 angles to match the reference math.
    k = np.arange(F, dtype=np.float32)[:, None]
    n = np.arange(N_FFT, dtype=np.float32)[None, :]
    ang = np.float32(2.0 * np.pi / N_FFT) * k * n
    cosk = np.cos(ang) * window[None, :]              # [F, 1024]
    sink = -np.sin(ang) * window[None, :]
    Bre = cosk.T.astype(np.float32)                   # [1024, F]
    Bim = sink.T.astype(np.float32)

    # chunk = g*16 + (s*2 + comp)*4 + q ; rows n = 2*(128q+p)+s
    bas_host = np.empty((128, 32, 128), np.float32)
    p = np.arange(128)
    for s in range(2):
        for comp in range(2):
            src = Bre if comp == 0 else Bim
            for g in range(2):
                for q in range(4):
                    ch = g * 16 + (s * 2 + comp) * 4 + q
                    bas_host[:, ch, :] = src[2 * (128 * q + p) + s,
                                             128 * g:128 * g + 128]
    bas_bf = bas_host.astype(ml_dtypes.bfloat16)

    # host freq k=256 (fp32 BLAS)
    hb = np.stack([Bre[:, 256], Bim[:, 256]], axis=1)  # [1024, 2]
    hout = np.empty((B, NF, 2), np.float32)
    for b in range(B):
        frames = np.lib.stride_tricks.as_strided(
            xp[b], (NF, N_FFT), (HOP * 4, 4))
        hout[b] = frames @ hb

    in_maps = []
    for core in range(NCORES):
        bsl = slice(B_PER_CORE * core, B_PER_CORE * (core + 1))
        sc = np.ascontiguousarray(lay_bf[bsl].transpose(2, 0, 1, 3))
        in_maps.append({"sig": sc, "basis": bas_bf})
    return in_maps, hout


def _assemble(results, prep):
    _, hout = prep
    dev = np.concatenate(
        [np.asarray(results.results[c]["out"]) for c in range(NCORES)],
        axis=0).astype(np.float32)            # [B, 2lu, 2g, 128, 2comp, NF]
    out = np.empty((B, NF, F, 2), np.float32)
    j = np.arange(128)
    for g in range(2):
        lo = dev[:, 0, g].transpose(0, 3, 1, 2)       # [B, NF, 128, 2]
        out[:, :, 128 * g:128 * g + 128, :] = lo
        up = dev[:, 1, g].transpose(0, 3, 1, 2)       # [B, NF, 128, 2]
        ks = 512 - (128 * g + j)                      # descending freqs
        out[:, :, ks, 0] = up[:, :, :, 0]
        out[:, :, ks, 1] = -up[:, :, :, 1]
    out[:, :, 256, 0] = hout[:, :, 0]
    out[:, :, 256, 1] = hout[:, :, 1]
    return out


def kernel(x, window):
    from concourse.bass_utils import run_bass_kernel_spmd

    if "nc" not in _CACHE:
        _CACHE["nc"] = _build_nc()
    nc = _CACHE["nc"]

    prep = _host_prep(np.asarray(x), np.asarray(window))
    res = run_bass_kernel_spmd(nc, prep[0], core_ids=list(range(NCORES)),
                               trace=False)
    return _assemble(res, prep)
